# revision 9
# baseline (speedup 1.0000x reference)
"""Trainium2 Bass kernel for nn_AdaptiveMobiusLayer.

Strategy (pure data parallel over tokens, 8 NeuronCores):
  - Flatten x [4, 4096, 1024] -> [16384, 1024] tokens; core c takes 2048
    consecutive tokens (= batch b = c//2, seq half c%2).
  - Host transposes each shard to [1024 feats, 2048 tokens] and ships it
    twice: bf16 (the in-SBUF `out` carrier) and fp8 (cycle-0 matmul moving
    operand), so the device does no cycle-0 conversions.
  - Feature-major tiles: every matmul keeps features on partitions (weights
    are natural [K, M] lhsT stationary operands, activations moving).
  - The seq-mean for the global context needs the partner core's partial sum:
    one tiny pairwise AllReduce ([128, 8] f32) overlapped with cycle-0 compute.
  - MLP matmuls run in fp8 DoubleRow (fp32 accumulation in PSUM); `out` is
    carried in bf16 so the twist-update tensor_tensor ops hit the DVE 2x_1p
    fast path.
  - All sigmoids are computed as 0.5 + 0.5*tanh(z/2); the coupling affine
    c0' + cmul'*tanh is folded into the partition-broadcast matmul (K=2
    stationary [cmul'; c0'], moving [tanh; ones]), so ACT only ever uses the
    gelu table (gelu/tanh share it -> no ACT_TABLE_LOAD thrash):
      coupling = (0.1 + 0.7*ar*gf_t) + (0.3*ar)*tanh((z4+b4)/2),
      gf_t = tanh((gz+gb3)/2).
  - DMAs are consolidated (one per x chunk / weight matrix / out chunk):
    dma_start issue costs ~0.6us of sequencer time each.
"""

import sys

sys.path.insert(0, "/opt/trn_rl_repo")

import numpy as np

B, S, DIM = 4, 4096, 1024
NCORES = 8
TOK = B * S // NCORES  # 2048 tokens per core
CHUNK = 512
NCHUNK = TOK // CHUNK  # 4
NUM_CYCLES = 3
BASE_COUPLING = 0.1

# feature-quarter twist:  out_new[t] = out[t] + sign[t] * c * out[(t+4) % 8]
# tiles 0..7 are 128-feature slabs; quarters = [t0 t1 | t2 t3 | t4 t5 | t6 t7]
TWIST_SIGN = [+1, +1, -1, -1, -1, -1, +1, +1]

_CACHE = {}


def _build_graph():
    import concourse.bass as bass
    import concourse.bacc as bacc
    import concourse.tile as tile
    import concourse.mybir as mybir

    f32 = mybir.dt.float32
    bf16 = mybir.dt.bfloat16
    AF = mybir.ActivationFunctionType
    ALU = mybir.AluOpType
    AX = mybir.AxisListType

    nc = bacc.Bacc(
        "TRN2", target_bir_lowering=False, debug=False, num_devices=NCORES
    )

    # ---- DRAM parameters (per-core shard; layouts prepared on host) ----
    x_d = nc.declare_dram_parameter("x", [DIM, TOK], bf16, isOutput=False)
    f8 = mybir.dt.float8e4
    DR = mybir.MatmulPerfMode.DoubleRow
    x8_d = nc.declare_dram_parameter("x8", [DIM, TOK], f8, isOutput=False)
    # coupling-net weights in fp8 (DoubleRow 2x matmul mode), packed in SBUF
    # tile layout, fo-major: row fo*128+p, col s*128+j == w[s*128+p, fo*128+j]
    w1_d = nc.declare_dram_parameter("cn_w1", [8 * 128, DIM], f8, isOutput=False)
    w2_d = nc.declare_dram_parameter("cn_w2", [4 * 128, DIM], f8, isOutput=False)
    w3_d = nc.declare_dram_parameter("cn_w3", [2 * 128, 512], f8, isOutput=False)
    w4_d = nc.declare_dram_parameter("cn_w4", [128, 2, 1], f8, isOutput=False)
    # all biases + scalars packed into one small tensor (single DMA):
    # cols 0-7 b1, 8-11 b2, 12-13 b3, 14-17 gb1, 18-19 gb2;
    # partition-0 scalars: [0,20]=b4/2 [0,21]=gb3/2 [0,22]=adaptive_range
    cst_d = nc.declare_dram_parameter("consts", [128, 23], f32, isOutput=False)
    gw1_d = nc.declare_dram_parameter("gc_w1", [DIM, 512], bf16, isOutput=False)
    gw2_d = nc.declare_dram_parameter("gc_w2", [512, 256], bf16, isOutput=False)
    gw3_d = nc.declare_dram_parameter("gc_w3", [256, 1], bf16, isOutput=False)
    out_d = nc.declare_dram_parameter("out", [DIM, TOK], bf16, isOutput=True)

    with tile.TileContext(nc) as tc:
        with (
            tc.tile_pool(name="const", bufs=1) as const,
            tc.tile_pool(name="work", bufs=2) as work,
            tc.tile_pool(name="psm", bufs=4, space="PSUM") as psm,
            tc.tile_pool(name="psx", bufs=1, space="PSUM") as psx,
            tc.tile_pool(name="psl4", bufs=1, space="PSUM") as psl4,
            tc.tile_pool(name="pscb", bufs=2, space="PSUM") as pscb,
            tc.tile_pool(name="dram", bufs=1, space="DRAM") as dram,
        ):
            # dma_start ISSUE costs ~0.6us on a sequencer, serially.  Only
            # sync has the fast HWDGE path for bulk; the ACT sequencer is idle
            # for the first ~15us, so it issues the const/weight DMAs, letting
            # sync start on x immediately.
            early = [0]

            def dma_rr(out, in_):
                if early[0] > 0:
                    early[0] -= 1
                    nc.scalar.dma_start(out=out, in_=in_)
                else:
                    nc.sync.dma_start(out=out, in_=in_)

            # bias/constant tile first (a late bias DMA gates every GELU on
            # the in-order ACT queue); one packed DMA.
            early[0] = 9  # cst + w1f halves + w4 + w2f + w3f + gw1/2/3
            cst = const.tile([128, 23], f32, tag="cst")
            dma_rr(cst[:], cst_d[:, :])
            b1 = cst[:, 0:8]
            b2 = cst[:, 8:12]
            b3 = cst[:, 12:14]
            gb1 = cst[:, 14:18]
            gb2 = cst[:, 18:20]
            b4h = cst[0:1, 20:21]   # b4 / 2 (host-packed)
            gb3h = cst[0:1, 21:22]  # gb3 / 2 (host-packed)
            ar = cst[0:1, 22:23]

            # coupling-net weights: one big tile per layer, 1-2 DMAs each
            w1f = const.tile([128, 8, 8, 128], f8, tag="w1f")  # [p, fo, s, j]
            for h in range(2):
                dma_rr(
                    w1f[:, 4 * h:4 * h + 4, :, :],
                    w1_d[4 * h * 128:(4 * h + 4) * 128, :].rearrange(
                        "(fo p) (s j) -> p fo s j", fo=4, s=8),
                )
            w4f = const.tile([128, 2, 1], f8, tag="w4f")
            dma_rr(w4f[:], w4_d[:, :, :])
            w2f = const.tile([128, 4, 8, 128], f8, tag="w2f")
            dma_rr(w2f[:], w2_d[:, :].rearrange(
                "(fo p) (s j) -> p fo s j", fo=4, s=8))
            w3f = const.tile([128, 2, 4, 128], f8, tag="w3f")
            dma_rr(w3f[:], w3_d[:, :].rearrange(
                "(fo p) (s j) -> p fo s j", fo=2, s=4))
            gw1 = const.tile([128, 8, 512], bf16, tag="gw1")  # [p, k, m]
            dma_rr(gw1[:], gw1_d[:, :].rearrange("(k p) m -> p k m", k=8))
            gw2 = const.tile([128, 4, 256], bf16, tag="gw2")
            dma_rr(gw2[:], gw2_d[:, :].rearrange("(k p) m -> p k m", k=4))
            gw3 = const.tile([128, 2, 1], bf16, tag="gw3")
            dma_rr(gw3[:], gw3_d[:, :].rearrange("(k p) m -> p k m", k=2))

            # x carrier (bf16, updated in place) + cycle-0 fp8 moving operand;
            # one DMA per chunk each, interleaved so chunk 0 lands first.
            xc = []   # [128, slab, tok] bf16 — `out` lives here
            x8c = []  # [128, slab, tok] fp8
            for c in range(NCHUNK):
                x8t = const.tile([128, 8, CHUNK], f8, tag=f"x8_{c}")
                nc.sync.dma_start(
                    out=x8t[:],
                    in_=x8_d[:, c * CHUNK:(c + 1) * CHUNK].rearrange(
                        "(t p) n -> p t n", t=8),
                )
                xt = const.tile([128, 8, CHUNK], bf16, tag=f"xc_{c}")
                nc.sync.dma_start(
                    out=xt[:],
                    in_=x_d[:, c * CHUNK:(c + 1) * CHUNK].rearrange(
                        "(t p) n -> p t n", t=8),
                )
                xc.append(xt)
                x8c.append(x8t)

            def out_bf(t, c):
                return xc[c][:, t, :]

            # per-chunk tanh tiles + coupling-broadcast operands.  The
            # coupling affine c0' + cmul'*th rides the PE broadcast as TWO
            # accumulating K=1 matmuls (engines cannot write partition 1, so
            # a single K=2 stationary cannot be built from runtime scalars):
            #   cb = (cmul'*ones) (x) th  +  (c0'*ones) (x) ones_row
            th_t = []
            for c in range(NCHUNK):
                th = const.tile([1, CHUNK], bf16, tag=f"th_{c}")
                th_t.append(th)
            cm_row = const.tile([1, 128], bf16, tag="cm_row")
            c0_row = const.tile([1, 128], bf16, tag="c0_row")
            ones = const.tile([1, 128], bf16, tag="ones")
            nc.vector.memset(ones[:], 1.0)
            ones_row = const.tile([1, CHUNK], bf16, tag="ones_row")
            nc.vector.memset(ones_row[:], 1.0)

            # ---------------- global-context partial sums + AllReduce -------
            # per-(tile, chunk) partial sums on DVE (idle during cycle 0);
            # emitted per-chunk so the in-order queue never blocks on later x.
            red = const.tile([128, 8, NCHUNK], f32, tag="gred")

            def reduce_chunk(c):
                for t in range(8):
                    nc.vector.tensor_reduce(
                        red[:, t, c:c + 1], out_bf(t, c), axis=AX.X, op=ALU.add
                    )

            gs = const.tile([128, 8], f32, tag="gs")

            def finish_gsum():
                for t in range(8):
                    nc.vector.tensor_reduce(
                        gs[:, t:t + 1], red[:, t, :], axis=AX.X, op=ALU.add
                    )

            cc_in = dram.tile([128, 8], f32, tag="cc_in")
            cc_out = dram.tile([128, 8], f32, tag="cc_out")
            gmean_f = const.tile([128, 8], f32, tag="gmean_f")
            gmean = const.tile([128, 8], bf16, tag="gmean")

            def do_collective():
                nc.sync.dma_start(out=cc_in[:], in_=gs[:])
                nc.gpsimd.collective_compute(
                    "AllReduce",
                    ALU.add,
                    ins=[cc_in.opt()],
                    outs=[cc_out.opt()],
                    replica_groups=[[0, 1], [2, 3], [4, 5], [6, 7]],
                )
                nc.sync.dma_start(out=gmean_f[:], in_=cc_out[:])
                nc.vector.tensor_copy(gmean[:], gmean_f[:])

            # ---------------- global net (emitted via hooks; see cycle 0) ---
            gc_tiles = {}

            def gc_stage1():
                # all 4 output-tile groups accumulate into one PSUM bank
                # (disjoint columns) -> a single GELU epilogue
                ps = psx.tile([128, 4], f32, tag="aux")
                for fo in range(4):
                    for k in range(8):
                        nc.tensor.matmul(
                            ps[:, fo:fo + 1],
                            gw1[:, k, fo * 128:(fo + 1) * 128],
                            gmean[:, k:k + 1], start=(k == 0), stop=(k == 7),
                        )
                # psum holds gc_w1.T @ sum(x); fold the 1/S mean + bias on DVE
                # (activation bias APs must be [P,1]; gb1 varies per column)
                z1 = work.tile([128, 4], f32, tag="z1")
                nc.vector.scalar_tensor_tensor(
                    z1[:], ps[:], 1.0 / S, gb1, ALU.mult, ALU.add
                )
                g1 = work.tile([128, 4], bf16, tag="g1")
                nc.scalar.activation(g1[:], z1[:], AF.Gelu)
                gc_tiles["g1"] = g1

            def gc_stage2():
                g1 = gc_tiles["g1"]
                ps = psx.tile([128, 2], f32, tag="aux")
                for fo in range(2):
                    for k in range(4):
                        nc.tensor.matmul(
                            ps[:, fo:fo + 1],
                            gw2[:, k, fo * 128:(fo + 1) * 128],
                            g1[:, k:k + 1], start=(k == 0), stop=(k == 3),
                        )
                z2 = work.tile([128, 2], f32, tag="z2")
                nc.vector.tensor_add(z2[:], ps[:], gb2)
                g2 = work.tile([128, 2], bf16, tag="g2")
                nc.scalar.activation(g2[:], z2[:], AF.Gelu)
                gc_tiles["g2"] = g2

            def gc_stage3():
                g2 = gc_tiles["g2"]
                ps = psx.tile([1, 1], f32, tag="aux")
                for k in range(2):
                    nc.tensor.matmul(
                        ps[:], gw3[:, k, :], g2[:, k:k + 1],
                        start=(k == 0), stop=(k == 1)
                    )
                # gf_t = tanh((gz + gb3)/2); sigmoid folded into the affine
                gft = const.tile([1, 1], f32, tag="gft")
                nc.scalar.activation(gft[:], ps[:], AF.Tanh, bias=gb3h, scale=0.5)

                # coupling = c0' + cmul' * tanh((z4+b4)/2)
                #   cmul' = 0.3*ar ;  c0' = 0.1 + 0.7*ar*gf_t
                cmul = const.tile([1, 1], f32, tag="cmul")
                nc.vector.tensor_scalar(cmul[:], ar, 0.3, None, ALU.mult)
                tmp0 = const.tile([1, 1], f32, tag="tmp0")
                nc.vector.tensor_scalar(tmp0[:], gft[:], 0.7, None, ALU.mult)
                c0 = const.tile([1, 1], f32, tag="c0")
                nc.vector.tensor_tensor(tmp0[:], ar, tmp0[:], ALU.mult)
                nc.vector.tensor_scalar(
                    c0[:], tmp0[:], BASE_COUPLING, None, ALU.add)
                # broadcast the two scalars across 128 cols (partition 0)
                nc.vector.tensor_scalar(
                    cm_row[:], ones[:], cmul[:], None, ALU.mult)
                nc.vector.tensor_scalar(
                    c0_row[:], ones[:], c0[:], None, ALU.mult)

            # ---------------- per-chunk building blocks ----------------
            pending_xb = [x8c[c] for c in range(NCHUNK)]

            def mlp_chunk(c, hooks=()):
                """coupling-net MLP on chunk c of `out`; returns chunk id.

                hooks: up to 3 closures emitted after L1/L2/L3 — used to slot
                the previous chunk's coupling-broadcast + twist update (and
                the tiny serial gc-net chain at cycle 0) into the queues at
                points where their ACT/DVE dependencies have had time to
                finish.
                """
                hooks = list(hooks) + [None] * 3
                xb = pending_xb[c]
                pending_xb[c] = None
                h1 = work.tile([128, 8, CHUNK], f8, tag="h1")
                for fo in range(8):
                    ps1 = psm.tile([128, CHUNK], f32, tag="mm")
                    for s in range(4):
                        nc.tensor.matmul(
                            ps1[:], w1f[:, fo, 2 * s:2 * s + 2, :],
                            xb[:, 2 * s:2 * s + 2, :],
                            start=(s == 0), stop=(s == 3), perf_mode=DR,
                        )
                    nc.scalar.activation(
                        h1[:, fo, :], ps1[:], AF.Gelu, bias=b1[:, fo:fo + 1])
                if hooks[0]:
                    hooks[0]()
                h2 = work.tile([128, 4, CHUNK], f8, tag="h2")
                for fo in range(4):
                    ps2 = psm.tile([128, CHUNK], f32, tag="mm")
                    for s in range(4):
                        nc.tensor.matmul(
                            ps2[:], w2f[:, fo, 2 * s:2 * s + 2, :],
                            h1[:, 2 * s:2 * s + 2, :],
                            start=(s == 0), stop=(s == 3), perf_mode=DR,
                        )
                    nc.scalar.activation(
                        h2[:, fo, :], ps2[:], AF.Gelu, bias=b2[:, fo:fo + 1])
                if hooks[1]:
                    hooks[1]()
                h3 = work.tile([128, 2, CHUNK], f8, tag="h3")
                for fo in range(2):
                    ps3 = psm.tile([128, CHUNK], f32, tag="mm")
                    for s in range(2):
                        nc.tensor.matmul(
                            ps3[:], w3f[:, fo, 2 * s:2 * s + 2, :],
                            h2[:, 2 * s:2 * s + 2, :],
                            start=(s == 0), stop=(s == 1), perf_mode=DR,
                        )
                    nc.scalar.activation(
                        h3[:, fo, :], ps3[:], AF.Gelu, bias=b3[:, fo:fo + 1])
                if hooks[2]:
                    hooks[2]()
                # L4: M=1 forbids the DoubleRow ldweights layout -> 2 plain
                # fp8 matmuls (ISA check s3_lw_dual_fp8_restrictions)
                ps4 = psl4.tile([1, CHUNK], f32, tag="l4")
                for s in range(2):
                    nc.tensor.matmul(
                        ps4[:], w4f[:, s, :], h3[:, s, :],
                        start=(s == 0), stop=(s == 1),
                    )
                # th = tanh((z4 + b4)/2); sigmoid folded into the coupling
                nc.scalar.activation(
                    th_t[c][:], ps4[:], AF.Tanh, bias=b4h, scale=0.5)
                return c

            def update_chunk(c, last, next_conv=False):
                """coupling broadcast + twist update (in place) on chunk c;
                one consolidated DMA out if last."""
                # cb[p, j] = cmul'*th[j] + c0'  via two accumulating K=1
                # matmuls (all operands on partition 0)
                cb = pscb.tile([128, CHUNK], f32, tag="cb")
                nc.tensor.matmul(
                    cb[:], cm_row[:], th_t[c][:, :], start=True, stop=False)
                nc.tensor.matmul(
                    cb[:], c0_row[:], ones_row[:], start=False, stop=True)
                # one bf16 SBUF copy so the twist tensor_tensor ops all run
                # in the DVE 2x_1p fast mode (PSUM/f32 operands disable it)
                cbb = work.tile([128, CHUNK], bf16, tag="cbb")
                nc.vector.tensor_copy(cbb[:], cb[:])
                xb_next = None
                for p in range(4):
                    t, u = p, p + 4
                    tmpa = work.tile([128, CHUNK], bf16, tag="twa")
                    tmpb = work.tile([128, CHUNK], bf16, tag="twb")
                    nc.vector.tensor_mul(tmpa[:], out_bf(u, c), cbb[:])
                    nc.vector.tensor_mul(tmpb[:], out_bf(t, c), cbb[:])
                    if TWIST_SIGN[t] > 0:
                        nc.vector.tensor_add(out_bf(t, c), out_bf(t, c), tmpa[:])
                    else:
                        nc.vector.tensor_sub(out_bf(t, c), out_bf(t, c), tmpa[:])
                    if TWIST_SIGN[u] > 0:
                        nc.vector.tensor_add(out_bf(u, c), out_bf(u, c), tmpb[:])
                    else:
                        nc.vector.tensor_sub(out_bf(u, c), out_bf(u, c), tmpb[:])
                    if next_conv:
                        # next cycle's fp8 conversion for this pair, emitted
                        # here so the in-order DVE queue releases the next
                        # chunk's matmul inputs as early as possible
                        if xb_next is None:
                            xb_next = work.tile([128, 8, CHUNK], f8, tag="xb")
                        nc.vector.tensor_copy(xb_next[:, t, :], out_bf(t, c))
                        nc.vector.tensor_copy(xb_next[:, u, :], out_bf(u, c))
                if last:
                    nc.sync.dma_start(
                        out=out_d[:, c * CHUNK:(c + 1) * CHUNK].rearrange(
                            "(t p) n -> p t n", t=8),
                        in_=xc[c][:],
                    )
                if next_conv and not last:
                    pending_xb[c] = xb_next

            # ---------------- main cycles ----------------
            # Cycle 0: chunk MLPs first; reductions emitted eagerly (chunk 3's
            # x lands while mlp1 computes); collective issued right after; the
            # serial gc-net stages ride mlp(3)'s hooks so each stage's
            # dependency has a full MLP layer of matmuls to finish; then the
            # couplings/updates.
            mlp_chunk(0)
            reduce_chunk(0)
            mlp_chunk(1)
            reduce_chunk(1)
            reduce_chunk(2)
            reduce_chunk(3)
            finish_gsum()
            do_collective()
            mlp_chunk(2)
            mlp_chunk(3, hooks=(gc_stage1, gc_stage2, gc_stage3))
            for c in range(NCHUNK):
                update_chunk(c, last=False, next_conv=True)
            # Cycles 1..2: chunk c's coupling-broadcast + twist update are
            # emitted inside chunk c+1's mlp (hook after L1) so the PE queue
            # never stalls waiting on the ACT->DVE coupling chain.
            for cyc in range(1, NUM_CYCLES):
                last = cyc == NUM_CYCLES - 1
                pend = [None]  # chunk awaiting update emission

                def upd_hook():
                    c = pend[0]
                    pend[0] = None
                    update_chunk(c, last, next_conv=not last)

                for c in range(NCHUNK):
                    hooks = (upd_hook,) if pend[0] is not None else ()
                    mlp_chunk(c, hooks=hooks)
                    pend[0] = c
                # last chunk of the cycle: no following mlp to hook into
                upd_hook()

    nc.compile()
    return nc


def _get_graph():
    if "nc" not in _CACHE:
        _CACHE["nc"] = _build_graph()
    return _CACHE["nc"]


def _pack_consts(inputs):
    cst = np.zeros((128, 23), np.float32)
    cst[:, 0:8] = np.asarray(inputs["cn_b1"], np.float32).reshape(8, 128).T
    cst[:, 8:12] = np.asarray(inputs["cn_b2"], np.float32).reshape(4, 128).T
    cst[:, 12:14] = np.asarray(inputs["cn_b3"], np.float32).reshape(2, 128).T
    cst[:, 14:18] = np.asarray(inputs["gc_b1"], np.float32).reshape(4, 128).T
    cst[:, 18:20] = np.asarray(inputs["gc_b2"], np.float32).reshape(2, 128).T
    cst[0, 20] = 0.5 * np.asarray(inputs["cn_b4"], np.float32).reshape(())
    cst[0, 21] = 0.5 * np.asarray(inputs["gc_b3"], np.float32).reshape(())
    cst[0, 22] = np.asarray(inputs["adaptive_range"], np.float32).reshape(())
    return cst


def _make_in_maps(inputs):
    import ml_dtypes

    bf = ml_dtypes.bfloat16
    f8 = ml_dtypes.float8_e4m3
    x = np.ascontiguousarray(inputs["x"], dtype=np.float32)
    xs = x.reshape(NCORES, TOK, DIM).transpose(0, 2, 1)  # [8, 1024, 2048]

    shared = {
        "cn_w1": np.ascontiguousarray(
            np.asarray(inputs["cn_w1"]).reshape(8, 128, 8, 128)
            .transpose(2, 1, 0, 3).reshape(8 * 128, DIM), dtype=f8),
        "cn_w2": np.ascontiguousarray(
            np.asarray(inputs["cn_w2"]).reshape(8, 128, 4, 128)
            .transpose(2, 1, 0, 3).reshape(4 * 128, DIM), dtype=f8),
        "cn_w3": np.ascontiguousarray(
            np.asarray(inputs["cn_w3"]).reshape(4, 128, 2, 128)
            .transpose(2, 1, 0, 3).reshape(2 * 128, 512), dtype=f8),
        "cn_w4": np.ascontiguousarray(
            np.asarray(inputs["cn_w4"]).reshape(2, 128).T.reshape(128, 2, 1),
            dtype=f8),
        "gc_w1": np.ascontiguousarray(inputs["gc_w1"], dtype=bf),
        "gc_w2": np.ascontiguousarray(inputs["gc_w2"], dtype=bf),
        "gc_w3": np.ascontiguousarray(inputs["gc_w3"].reshape(256, 1), dtype=bf),
        "consts": _pack_consts(inputs),
    }
    in_maps = []
    for c in range(NCORES):
        m = dict(shared)
        m["x"] = np.ascontiguousarray(xs[c], dtype=bf)
        m["x8"] = np.ascontiguousarray(xs[c], dtype=f8)
        in_maps.append(m)
    return in_maps


def _run(inputs, trace=False):
    from concourse.bass_utils import run_bass_kernel_spmd

    nc = _get_graph()
    in_maps = _make_in_maps(inputs)
    res = run_bass_kernel_spmd(
        nc, in_maps, core_ids=list(range(NCORES)), trace=trace
    )
    outs = np.stack(
        [np.asarray(res.results[c]["out"]).astype(np.float32).T
         for c in range(NCORES)], axis=0
    )  # [8, 2048, 1024]
    full = outs.reshape(B, S, DIM).astype(np.float32)
    return full, res


def kernel(**inputs) -> np.ndarray:
    out, _ = _run(inputs, trace=False)
    return out


# revision 11
# speedup vs baseline: 1.0365x; 1.0365x over previous
"""Trainium2 Bass kernel for nn_AdaptiveMobiusLayer.

Strategy (pure data parallel over tokens, 8 NeuronCores):
  - Flatten x [4, 4096, 1024] -> [16384, 1024] tokens; core c takes 2048
    consecutive tokens (= batch b = c//2, seq half c%2).
  - Host transposes each shard to [1024 feats, 2048 tokens] and ships it
    twice: bf16 (the in-SBUF `out` carrier) and fp8 (cycle-0 matmul moving
    operand), so the device does no cycle-0 conversions.
  - Feature-major tiles: every matmul keeps features on partitions (weights
    are natural [K, M] lhsT stationary operands, activations moving).
  - The seq-mean for the global context needs the partner core's partial sum:
    one tiny pairwise AllReduce ([128, 8] f32) overlapped with cycle-0 compute.
  - MLP matmuls run in fp8 DoubleRow (fp32 accumulation in PSUM); `out` is
    carried in bf16 so the twist-update tensor_tensor ops hit the DVE 2x_1p
    fast path.
  - All sigmoids are computed as 0.5 + 0.5*tanh(z/2); the coupling affine
    c0' + cmul'*tanh is folded into the partition-broadcast matmul (K=2
    stationary [cmul'; c0'], moving [tanh; ones]), so ACT only ever uses the
    gelu table (gelu/tanh share it -> no ACT_TABLE_LOAD thrash):
      coupling = (0.1 + 0.7*ar*gf_t) + (0.3*ar)*tanh((z4+b4)/2),
      gf_t = tanh((gz+gb3)/2).
  - DMAs are consolidated (one per x chunk / weight matrix / out chunk):
    dma_start issue costs ~0.6us of sequencer time each.
"""

import sys

sys.path.insert(0, "/opt/trn_rl_repo")

import numpy as np

B, S, DIM = 4, 4096, 1024
NCORES = 8
TOK = B * S // NCORES  # 2048 tokens per core
CHUNK = 512
NCHUNK = TOK // CHUNK  # 4
NUM_CYCLES = 3
BASE_COUPLING = 0.1

# feature-quarter twist:  out_new[t] = out[t] + sign[t] * c * out[(t+4) % 8]
# tiles 0..7 are 128-feature slabs; quarters = [t0 t1 | t2 t3 | t4 t5 | t6 t7]
TWIST_SIGN = [+1, +1, -1, -1, -1, -1, +1, +1]

_CACHE = {}


def _build_graph():
    import concourse.bass as bass
    import concourse.bacc as bacc
    import concourse.tile as tile
    import concourse.mybir as mybir

    f32 = mybir.dt.float32
    bf16 = mybir.dt.bfloat16
    AF = mybir.ActivationFunctionType
    ALU = mybir.AluOpType
    AX = mybir.AxisListType

    nc = bacc.Bacc(
        "TRN2", target_bir_lowering=False, debug=False, num_devices=NCORES
    )

    # ---- DRAM parameters (per-core shard; ALL tensors are host-packed into
    # their exact SBUF tile layouts so every DMA is partition-contiguous:
    # one DMA = one HW channel, and small strided rows run descriptor-bound)
    f8 = mybir.dt.float8e4
    DR = mybir.MatmulPerfMode.DoubleRow
    # x carrier / fp8 copy: [p, chunk, slab, tok]
    x_d = nc.declare_dram_parameter(
        "x", [128, NCHUNK, 8, CHUNK], bf16, isOutput=False)
    x8_d = nc.declare_dram_parameter(
        "x8", [128, NCHUNK, 8, CHUNK], f8, isOutput=False)
    # coupling-net weights fp8 (DoubleRow layout): [p, fo, s, j] with
    # w1f[p, fo, s, j] == w1[s*128+p, fo*128+j]
    w1_d = nc.declare_dram_parameter("cn_w1", [128, 8, 8, 128], f8, isOutput=False)
    w2_d = nc.declare_dram_parameter("cn_w2", [128, 4, 8, 128], f8, isOutput=False)
    w3_d = nc.declare_dram_parameter("cn_w3", [128, 2, 4, 128], f8, isOutput=False)
    w4_d = nc.declare_dram_parameter("cn_w4", [128, 2, 1], f8, isOutput=False)
    # all biases + scalars packed into one small tensor (single DMA):
    # cols 0-7 b1, 8-11 b2, 12-13 b3, 14-17 gb1, 18-19 gb2;
    # partition-0 scalars: [0,20]=b4/2 [0,21]=gb3/2 [0,22]=adaptive_range
    cst_d = nc.declare_dram_parameter("consts", [128, 23], f32, isOutput=False)
    # global-net weights bf16: [p, k, m] with gw1[p, k, m] == gc_w1[k*128+p, m]
    gw1_d = nc.declare_dram_parameter("gc_w1", [128, 8, 512], bf16, isOutput=False)
    gw2_d = nc.declare_dram_parameter("gc_w2", [128, 4, 256], bf16, isOutput=False)
    gw3_d = nc.declare_dram_parameter("gc_w3", [128, 2, 1], bf16, isOutput=False)
    out_d = nc.declare_dram_parameter(
        "out", [128, NCHUNK, 8, CHUNK], bf16, isOutput=True)

    with tile.TileContext(nc) as tc:
        with (
            tc.tile_pool(name="const", bufs=1) as const,
            tc.tile_pool(name="work", bufs=2) as work,
            tc.tile_pool(name="psm", bufs=4, space="PSUM") as psm,
            tc.tile_pool(name="psx", bufs=1, space="PSUM") as psx,
            tc.tile_pool(name="psl4", bufs=1, space="PSUM") as psl4,
            tc.tile_pool(name="pscb", bufs=2, space="PSUM") as pscb,
            tc.tile_pool(name="dram", bufs=1, space="DRAM") as dram,
        ):
            # dma_start ISSUE costs ~0.6us on a sequencer, serially.  Only
            # sync has the fast HWDGE path for bulk; the ACT sequencer is idle
            # for the first ~15us, so it issues the const/weight DMAs, letting
            # sync start on x immediately.
            early = [0]

            def dma_rr(out, in_):
                if early[0] > 0:
                    early[0] -= 1
                    nc.scalar.dma_start(out=out, in_=in_)
                else:
                    nc.sync.dma_start(out=out, in_=in_)

            # bias/constant tile first (a late bias DMA gates every GELU on
            # the in-order ACT queue); one packed DMA.
            early[0] = 9  # cst + w1f halves + w4 + w2f + w3f + gw1/2/3
            cst = const.tile([128, 23], f32, tag="cst")
            dma_rr(cst[:], cst_d[:, :])
            b1 = cst[:, 0:8]
            b2 = cst[:, 8:12]
            b3 = cst[:, 12:14]
            gb1 = cst[:, 14:18]
            gb2 = cst[:, 18:20]
            b4h = cst[0:1, 20:21]   # b4 / 2 (host-packed)
            gb3h = cst[0:1, 21:22]  # gb3 / 2 (host-packed)
            ar = cst[0:1, 22:23]

            # coupling-net weights: one big tile per layer, 1-2 DMAs each
            w1f = const.tile([128, 8, 8, 128], f8, tag="w1f")  # [p, fo, s, j]
            for h in range(2):
                dma_rr(w1f[:, 4 * h:4 * h + 4, :, :],
                       w1_d[:, 4 * h:4 * h + 4, :, :])
            w4f = const.tile([128, 2, 1], f8, tag="w4f")
            dma_rr(w4f[:], w4_d[:, :, :])
            w2f = const.tile([128, 4, 8, 128], f8, tag="w2f")
            dma_rr(w2f[:], w2_d[:, :, :, :])
            w3f = const.tile([128, 2, 4, 128], f8, tag="w3f")
            dma_rr(w3f[:], w3_d[:, :, :, :])
            gw1 = const.tile([128, 8, 512], bf16, tag="gw1")  # [p, k, m]
            dma_rr(gw1[:], gw1_d[:, :, :])
            gw2 = const.tile([128, 4, 256], bf16, tag="gw2")
            dma_rr(gw2[:], gw2_d[:, :, :])
            gw3 = const.tile([128, 2, 1], bf16, tag="gw3")
            dma_rr(gw3[:], gw3_d[:, :, :])

            # x carrier (bf16, updated in place) + cycle-0 fp8 moving operand;
            # two DMAs per chunk each (slab halves -> separate HW channels),
            # interleaved so chunk 0 lands first.
            xc = []   # [128, slab, tok] bf16 — `out` lives here
            x8c = []  # [128, slab, tok] fp8
            for c in range(NCHUNK):
                x8t = const.tile([128, 8, CHUNK], f8, tag=f"x8_{c}")
                xt = const.tile([128, 8, CHUNK], bf16, tag=f"xc_{c}")
                for h in range(2):
                    sl = slice(4 * h, 4 * h + 4)
                    nc.sync.dma_start(out=x8t[:, sl, :], in_=x8_d[:, c, sl, :])
                    nc.sync.dma_start(out=xt[:, sl, :], in_=x_d[:, c, sl, :])
                xc.append(xt)
                x8c.append(x8t)

            def out_bf(t, c):
                return xc[c][:, t, :]

            # per-chunk tanh tiles + coupling-broadcast operands.  The
            # coupling affine c0' + cmul'*th rides the PE broadcast as TWO
            # accumulating K=1 matmuls (engines cannot write partition 1, so
            # a single K=2 stationary cannot be built from runtime scalars):
            #   cb = (cmul'*ones) (x) th  +  (c0'*ones) (x) ones_row
            th_t = []
            for c in range(NCHUNK):
                th = const.tile([1, CHUNK], bf16, tag=f"th_{c}")
                th_t.append(th)
            cm_row = const.tile([1, 128], bf16, tag="cm_row")
            c0_row = const.tile([1, 128], bf16, tag="c0_row")
            ones = const.tile([1, 128], bf16, tag="ones")
            nc.vector.memset(ones[:], 1.0)
            ones_row = const.tile([1, CHUNK], bf16, tag="ones_row")
            nc.vector.memset(ones_row[:], 1.0)

            # ---------------- global-context partial sums + AllReduce -------
            # per-(tile, chunk) partial sums on DVE (idle during cycle 0);
            # emitted per-chunk so the in-order queue never blocks on later x.
            red = const.tile([128, 8, NCHUNK], f32, tag="gred")

            def reduce_chunk(c):
                for t in range(8):
                    nc.vector.tensor_reduce(
                        red[:, t, c:c + 1], out_bf(t, c), axis=AX.X, op=ALU.add
                    )

            gs = const.tile([128, 8], f32, tag="gs")

            def finish_gsum():
                for t in range(8):
                    nc.vector.tensor_reduce(
                        gs[:, t:t + 1], red[:, t, :], axis=AX.X, op=ALU.add
                    )

            cc_in = dram.tile([128, 8], f32, tag="cc_in")
            cc_out = dram.tile([128, 8], f32, tag="cc_out")
            gmean_f = const.tile([128, 8], f32, tag="gmean_f")
            gmean = const.tile([128, 8], bf16, tag="gmean")

            def do_collective():
                nc.sync.dma_start(out=cc_in[:], in_=gs[:])
                nc.gpsimd.collective_compute(
                    "AllReduce",
                    ALU.add,
                    ins=[cc_in.opt()],
                    outs=[cc_out.opt()],
                    replica_groups=[[0, 1], [2, 3], [4, 5], [6, 7]],
                )
                nc.sync.dma_start(out=gmean_f[:], in_=cc_out[:])
                nc.vector.tensor_copy(gmean[:], gmean_f[:])

            # ---------------- global net (emitted via hooks; see cycle 0) ---
            gc_tiles = {}

            def gc_stage1():
                # all 4 output-tile groups accumulate into one PSUM bank
                # (disjoint columns) -> a single GELU epilogue
                ps = psx.tile([128, 4], f32, tag="aux")
                for fo in range(4):
                    for k in range(8):
                        nc.tensor.matmul(
                            ps[:, fo:fo + 1],
                            gw1[:, k, fo * 128:(fo + 1) * 128],
                            gmean[:, k:k + 1], start=(k == 0), stop=(k == 7),
                        )
                # psum holds gc_w1.T @ sum(x); fold the 1/S mean + bias on DVE
                # (activation bias APs must be [P,1]; gb1 varies per column)
                z1 = work.tile([128, 4], f32, tag="z1")
                nc.vector.scalar_tensor_tensor(
                    z1[:], ps[:], 1.0 / S, gb1, ALU.mult, ALU.add
                )
                g1 = work.tile([128, 4], bf16, tag="g1")
                nc.scalar.activation(g1[:], z1[:], AF.Gelu)
                gc_tiles["g1"] = g1

            def gc_stage2():
                g1 = gc_tiles["g1"]
                ps = psx.tile([128, 2], f32, tag="aux")
                for fo in range(2):
                    for k in range(4):
                        nc.tensor.matmul(
                            ps[:, fo:fo + 1],
                            gw2[:, k, fo * 128:(fo + 1) * 128],
                            g1[:, k:k + 1], start=(k == 0), stop=(k == 3),
                        )
                z2 = work.tile([128, 2], f32, tag="z2")
                nc.vector.tensor_add(z2[:], ps[:], gb2)
                g2 = work.tile([128, 2], bf16, tag="g2")
                nc.scalar.activation(g2[:], z2[:], AF.Gelu)
                gc_tiles["g2"] = g2

            def gc_stage3():
                g2 = gc_tiles["g2"]
                ps = psx.tile([1, 1], f32, tag="aux")
                for k in range(2):
                    nc.tensor.matmul(
                        ps[:], gw3[:, k, :], g2[:, k:k + 1],
                        start=(k == 0), stop=(k == 1)
                    )
                # gf_t = tanh((gz + gb3)/2); sigmoid folded into the affine
                gft = const.tile([1, 1], f32, tag="gft")
                nc.scalar.activation(gft[:], ps[:], AF.Tanh, bias=gb3h, scale=0.5)

                # coupling = c0' + cmul' * tanh((z4+b4)/2)
                #   cmul' = 0.3*ar ;  c0' = 0.1 + 0.7*ar*gf_t
                cmul = const.tile([1, 1], f32, tag="cmul")
                nc.vector.tensor_scalar(cmul[:], ar, 0.3, None, ALU.mult)
                tmp0 = const.tile([1, 1], f32, tag="tmp0")
                nc.vector.tensor_scalar(tmp0[:], gft[:], 0.7, None, ALU.mult)
                c0 = const.tile([1, 1], f32, tag="c0")
                nc.vector.tensor_tensor(tmp0[:], ar, tmp0[:], ALU.mult)
                nc.vector.tensor_scalar(
                    c0[:], tmp0[:], BASE_COUPLING, None, ALU.add)
                # broadcast the two scalars across 128 cols (partition 0)
                nc.vector.tensor_scalar(
                    cm_row[:], ones[:], cmul[:], None, ALU.mult)
                nc.vector.tensor_scalar(
                    c0_row[:], ones[:], c0[:], None, ALU.mult)

            # ---------------- per-chunk building blocks ----------------
            pending_xb = [x8c[c] for c in range(NCHUNK)]

            def mlp_chunk(c, hooks=()):
                """coupling-net MLP on chunk c of `out`; returns chunk id.

                hooks: up to 3 closures emitted after L1/L2/L3 — used to slot
                the previous chunk's coupling-broadcast + twist update (and
                the tiny serial gc-net chain at cycle 0) into the queues at
                points where their ACT/DVE dependencies have had time to
                finish.
                """
                hooks = list(hooks) + [None] * 3
                xb = pending_xb[c]
                pending_xb[c] = None
                h1 = work.tile([128, 8, CHUNK], f8, tag="h1")
                for fo in range(8):
                    ps1 = psm.tile([128, CHUNK], f32, tag="mm")
                    for s in range(4):
                        nc.tensor.matmul(
                            ps1[:], w1f[:, fo, 2 * s:2 * s + 2, :],
                            xb[:, 2 * s:2 * s + 2, :],
                            start=(s == 0), stop=(s == 3), perf_mode=DR,
                        )
                    nc.scalar.activation(
                        h1[:, fo, :], ps1[:], AF.Gelu, bias=b1[:, fo:fo + 1])
                if hooks[0]:
                    hooks[0]()
                h2 = work.tile([128, 4, CHUNK], f8, tag="h2")
                for fo in range(4):
                    ps2 = psm.tile([128, CHUNK], f32, tag="mm")
                    for s in range(4):
                        nc.tensor.matmul(
                            ps2[:], w2f[:, fo, 2 * s:2 * s + 2, :],
                            h1[:, 2 * s:2 * s + 2, :],
                            start=(s == 0), stop=(s == 3), perf_mode=DR,
                        )
                    nc.scalar.activation(
                        h2[:, fo, :], ps2[:], AF.Gelu, bias=b2[:, fo:fo + 1])
                if hooks[1]:
                    hooks[1]()
                h3 = work.tile([128, 2, CHUNK], f8, tag="h3")
                for fo in range(2):
                    ps3 = psm.tile([128, CHUNK], f32, tag="mm")
                    for s in range(2):
                        nc.tensor.matmul(
                            ps3[:], w3f[:, fo, 2 * s:2 * s + 2, :],
                            h2[:, 2 * s:2 * s + 2, :],
                            start=(s == 0), stop=(s == 1), perf_mode=DR,
                        )
                    nc.scalar.activation(
                        h3[:, fo, :], ps3[:], AF.Gelu, bias=b3[:, fo:fo + 1])
                if hooks[2]:
                    hooks[2]()
                # L4: M=1 forbids the DoubleRow ldweights layout -> 2 plain
                # fp8 matmuls (ISA check s3_lw_dual_fp8_restrictions)
                ps4 = psl4.tile([1, CHUNK], f32, tag="l4")
                for s in range(2):
                    nc.tensor.matmul(
                        ps4[:], w4f[:, s, :], h3[:, s, :],
                        start=(s == 0), stop=(s == 1),
                    )
                # th = tanh((z4 + b4)/2); sigmoid folded into the coupling
                nc.scalar.activation(
                    th_t[c][:], ps4[:], AF.Tanh, bias=b4h, scale=0.5)
                return c

            def update_chunk(c, last, next_conv=False):
                """coupling broadcast + twist update (in place) on chunk c;
                one consolidated DMA out if last."""
                # cb[p, j] = cmul'*th[j] + c0'  via two accumulating K=1
                # matmuls (all operands on partition 0)
                cb = pscb.tile([128, CHUNK], f32, tag="cb")
                nc.tensor.matmul(
                    cb[:], cm_row[:], th_t[c][:, :], start=True, stop=False)
                nc.tensor.matmul(
                    cb[:], c0_row[:], ones_row[:], start=False, stop=True)
                # one bf16 SBUF copy so the twist tensor_tensor ops all run
                # in the DVE 2x_1p fast mode (PSUM/f32 operands disable it)
                cbb = work.tile([128, CHUNK], bf16, tag="cbb")
                nc.vector.tensor_copy(cbb[:], cb[:])
                xb_next = None
                for p in range(4):
                    t, u = p, p + 4
                    tmpa = work.tile([128, CHUNK], bf16, tag="twa")
                    tmpb = work.tile([128, CHUNK], bf16, tag="twb")
                    nc.vector.tensor_mul(tmpa[:], out_bf(u, c), cbb[:])
                    nc.vector.tensor_mul(tmpb[:], out_bf(t, c), cbb[:])
                    if TWIST_SIGN[t] > 0:
                        nc.vector.tensor_add(out_bf(t, c), out_bf(t, c), tmpa[:])
                    else:
                        nc.vector.tensor_sub(out_bf(t, c), out_bf(t, c), tmpa[:])
                    if TWIST_SIGN[u] > 0:
                        nc.vector.tensor_add(out_bf(u, c), out_bf(u, c), tmpb[:])
                    else:
                        nc.vector.tensor_sub(out_bf(u, c), out_bf(u, c), tmpb[:])
                    if next_conv:
                        # next cycle's fp8 conversion for this pair, emitted
                        # here so the in-order DVE queue releases the next
                        # chunk's matmul inputs as early as possible
                        if xb_next is None:
                            xb_next = work.tile([128, 8, CHUNK], f8, tag="xb")
                        nc.vector.tensor_copy(xb_next[:, t, :], out_bf(t, c))
                        nc.vector.tensor_copy(xb_next[:, u, :], out_bf(u, c))
                    if last and p % 2 == 1:
                        # DMA out slabs {p-1, p, p+3, p+4} as soon as their
                        # twist adds retire (shrinks the end-of-kernel tail)
                        for tt in (p - 1, p):
                            nc.sync.dma_start(
                                out=out_d[:, c, tt:tt + 4 + 1:4, :],
                                in_=xc[c][:, tt:tt + 4 + 1:4, :],
                            )
                if next_conv and not last:
                    pending_xb[c] = xb_next

            # ---------------- main cycles ----------------
            # Cycle 0: chunk MLPs first; reductions emitted eagerly (chunk 3's
            # x lands while mlp1 computes); collective issued right after; the
            # serial gc-net stages ride mlp(3)'s hooks so each stage's
            # dependency has a full MLP layer of matmuls to finish; then the
            # couplings/updates.
            mlp_chunk(0)
            reduce_chunk(0)
            mlp_chunk(1)
            reduce_chunk(1)
            reduce_chunk(2)
            reduce_chunk(3)
            finish_gsum()
            do_collective()
            mlp_chunk(2)
            mlp_chunk(3, hooks=(gc_stage1, gc_stage2, gc_stage3))
            for c in range(NCHUNK):
                update_chunk(c, last=False, next_conv=True)
            # Cycles 1..2: chunk c's coupling-broadcast + twist update are
            # emitted inside chunk c+1's mlp (hook after L1) so the PE queue
            # never stalls waiting on the ACT->DVE coupling chain.
            for cyc in range(1, NUM_CYCLES):
                last = cyc == NUM_CYCLES - 1
                pend = [None]  # chunk awaiting update emission

                def upd_hook():
                    c = pend[0]
                    pend[0] = None
                    update_chunk(c, last, next_conv=not last)

                for c in range(NCHUNK):
                    hooks = (upd_hook,) if pend[0] is not None else ()
                    mlp_chunk(c, hooks=hooks)
                    pend[0] = c
                # last chunk of the cycle: no following mlp to hook into
                upd_hook()

    nc.compile()
    return nc


def _get_graph():
    if "nc" not in _CACHE:
        _CACHE["nc"] = _build_graph()
    return _CACHE["nc"]


def _pack_consts(inputs):
    cst = np.zeros((128, 23), np.float32)
    cst[:, 0:8] = np.asarray(inputs["cn_b1"], np.float32).reshape(8, 128).T
    cst[:, 8:12] = np.asarray(inputs["cn_b2"], np.float32).reshape(4, 128).T
    cst[:, 12:14] = np.asarray(inputs["cn_b3"], np.float32).reshape(2, 128).T
    cst[:, 14:18] = np.asarray(inputs["gc_b1"], np.float32).reshape(4, 128).T
    cst[:, 18:20] = np.asarray(inputs["gc_b2"], np.float32).reshape(2, 128).T
    cst[0, 20] = 0.5 * np.asarray(inputs["cn_b4"], np.float32).reshape(())
    cst[0, 21] = 0.5 * np.asarray(inputs["gc_b3"], np.float32).reshape(())
    cst[0, 22] = np.asarray(inputs["adaptive_range"], np.float32).reshape(())
    return cst


def _make_in_maps(inputs):
    import ml_dtypes

    bf = ml_dtypes.bfloat16
    f8 = ml_dtypes.float8_e4m3
    x = np.ascontiguousarray(inputs["x"], dtype=np.float32)
    # [core, p, chunk, slab, tok] — exact SBUF tile layout, so every DMA is
    # partition-contiguous: x[b, s, d] with token j = chunk*512+tok,
    # feature f = slab*128+p
    xs = (x.reshape(NCORES, NCHUNK, CHUNK, 8, 128)
          .transpose(0, 4, 1, 3, 2))  # [8, 128, 4, 8, 512]
    xs = np.ascontiguousarray(xs)

    def pack_w(w, nfo):
        # [s*128+p, fo*128+j] -> [p, fo, s, j]
        kin = w.shape[0]
        return np.ascontiguousarray(
            np.asarray(w).reshape(kin // 128, 128, nfo, 128)
            .transpose(1, 2, 0, 3), dtype=f8)

    def pack_gw(w):
        # [k*128+p, m] -> [p, k, m]
        kin, m = w.shape
        return np.ascontiguousarray(
            np.asarray(w).reshape(kin // 128, 128, m).transpose(1, 0, 2),
            dtype=bf)

    shared = {
        "cn_w1": pack_w(inputs["cn_w1"], 8),
        "cn_w2": pack_w(inputs["cn_w2"], 4),
        "cn_w3": pack_w(inputs["cn_w3"], 2),
        "cn_w4": np.ascontiguousarray(
            np.asarray(inputs["cn_w4"]).reshape(2, 128).T.reshape(128, 2, 1),
            dtype=f8),
        "gc_w1": pack_gw(np.asarray(inputs["gc_w1"])),
        "gc_w2": pack_gw(np.asarray(inputs["gc_w2"])),
        "gc_w3": pack_gw(np.asarray(inputs["gc_w3"]).reshape(256, 1)),
        "consts": _pack_consts(inputs),
    }
    in_maps = []
    for c in range(NCORES):
        m = dict(shared)
        m["x"] = xs[c].astype(bf)
        m["x8"] = xs[c].astype(f8)
        in_maps.append(m)
    return in_maps


def _run(inputs, trace=False):
    from concourse.bass_utils import run_bass_kernel_spmd

    nc = _get_graph()
    in_maps = _make_in_maps(inputs)
    res = run_bass_kernel_spmd(
        nc, in_maps, core_ids=list(range(NCORES)), trace=trace
    )
    # out[p, chunk, slab, tok] -> [token = chunk*512+tok, feat = slab*128+p]
    outs = np.stack(
        [np.asarray(res.results[c]["out"]).astype(np.float32)
         .transpose(1, 3, 2, 0).reshape(TOK, DIM)
         for c in range(NCORES)], axis=0
    )  # [8, 2048, 1024]
    full = outs.reshape(B, S, DIM).astype(np.float32)
    return full, res


def kernel(**inputs) -> np.ndarray:
    out, _ = _run(inputs, trace=False)
    return out


# revision 13
# speedup vs baseline: 1.0522x; 1.0151x over previous
"""Trainium2 Bass kernel for nn_AdaptiveMobiusLayer.

Strategy (pure data parallel over tokens, 8 NeuronCores):
  - Flatten x [4, 4096, 1024] -> [16384, 1024] tokens; core c takes 2048
    consecutive tokens (= batch b = c//2, seq half c%2).
  - Host transposes each shard to [1024 feats, 2048 tokens] and ships it
    twice: bf16 (the in-SBUF `out` carrier) and fp8 (cycle-0 matmul moving
    operand), so the device does no cycle-0 conversions.
  - Feature-major tiles: every matmul keeps features on partitions (weights
    are natural [K, M] lhsT stationary operands, activations moving).
  - The seq-mean for the global context needs the partner core's partial sum:
    one tiny pairwise AllReduce ([128, 8] f32) overlapped with cycle-0 compute.
  - MLP matmuls run in fp8 DoubleRow (fp32 accumulation in PSUM); `out` is
    carried in bf16 so the twist-update tensor_tensor ops hit the DVE 2x_1p
    fast path.
  - All sigmoids are computed as 0.5 + 0.5*tanh(z/2); the coupling affine
    c0' + cmul'*tanh is folded into the partition-broadcast matmul (K=2
    stationary [cmul'; c0'], moving [tanh; ones]), so ACT only ever uses the
    gelu table (gelu/tanh share it -> no ACT_TABLE_LOAD thrash):
      coupling = (0.1 + 0.7*ar*gf_t) + (0.3*ar)*tanh((z4+b4)/2),
      gf_t = tanh((gz+gb3)/2).
  - DMAs are consolidated (one per x chunk / weight matrix / out chunk):
    dma_start issue costs ~0.6us of sequencer time each.
"""

import sys

sys.path.insert(0, "/opt/trn_rl_repo")

import numpy as np

B, S, DIM = 4, 4096, 1024
NCORES = 8
TOK = B * S // NCORES  # 2048 tokens per core
CHUNK = 512
NCHUNK = TOK // CHUNK  # 4
NUM_CYCLES = 3
BASE_COUPLING = 0.1

# feature-quarter twist:  out_new[t] = out[t] + sign[t] * c * out[(t+4) % 8]
# tiles 0..7 are 128-feature slabs; quarters = [t0 t1 | t2 t3 | t4 t5 | t6 t7]
TWIST_SIGN = [+1, +1, -1, -1, -1, -1, +1, +1]

_CACHE = {}


def _build_graph():
    import concourse.bass as bass
    import concourse.bacc as bacc
    import concourse.tile as tile
    import concourse.mybir as mybir

    f32 = mybir.dt.float32
    bf16 = mybir.dt.bfloat16
    AF = mybir.ActivationFunctionType
    ALU = mybir.AluOpType
    AX = mybir.AxisListType

    nc = bacc.Bacc(
        "TRN2", target_bir_lowering=False, debug=False, num_devices=NCORES
    )

    # ---- DRAM parameters (per-core shard; ALL tensors are host-packed into
    # their exact SBUF tile layouts so every DMA is partition-contiguous:
    # one DMA = one HW channel, and small strided rows run descriptor-bound)
    f8 = mybir.dt.float8e4
    DR = mybir.MatmulPerfMode.DoubleRow
    # x carrier / fp8 copy: [p, chunk, slab, tok]
    x_d = nc.declare_dram_parameter(
        "x", [128, NCHUNK, 8, CHUNK], bf16, isOutput=False)
    x8_d = nc.declare_dram_parameter(
        "x8", [128, NCHUNK, 8, CHUNK], f8, isOutput=False)
    # coupling-net weights fp8 (DoubleRow layout): [p, fo, s, j] with
    # w1f[p, fo, s, j] == w1[s*128+p, fo*128+j]
    w1_d = nc.declare_dram_parameter("cn_w1", [128, 8, 8, 128], f8, isOutput=False)
    w2_d = nc.declare_dram_parameter("cn_w2", [128, 4, 8, 128], f8, isOutput=False)
    w3_d = nc.declare_dram_parameter("cn_w3", [128, 2, 4, 128], f8, isOutput=False)
    w4_d = nc.declare_dram_parameter("cn_w4", [128, 2, 1], f8, isOutput=False)
    # all biases + scalars packed into one small tensor (single DMA):
    # cols 0-7 b1, 8-11 b2, 12-13 b3, 14-17 gb1, 18-19 gb2;
    # partition-0 scalars: [0,20]=b4/2 [0,21]=gb3/2 [0,22]=adaptive_range
    cst_d = nc.declare_dram_parameter("consts", [128, 23], f32, isOutput=False)
    # global-net weights bf16: [p, k, m] with gw1[p, k, m] == gc_w1[k*128+p, m]
    gw1_d = nc.declare_dram_parameter("gc_w1", [128, 8, 512], bf16, isOutput=False)
    gw2_d = nc.declare_dram_parameter("gc_w2", [128, 4, 256], bf16, isOutput=False)
    gw3_d = nc.declare_dram_parameter("gc_w3", [128, 2, 1], bf16, isOutput=False)
    out_d = nc.declare_dram_parameter(
        "out", [128, NCHUNK, 8, CHUNK], bf16, isOutput=True)

    with tile.TileContext(nc) as tc:
        with (
            tc.tile_pool(name="const", bufs=1) as const,
            tc.tile_pool(name="work", bufs=2) as work,
            tc.tile_pool(name="xbp", bufs=3) as xbp,
            tc.tile_pool(name="psm", bufs=4, space="PSUM") as psm,
            tc.tile_pool(name="psx", bufs=1, space="PSUM") as psx,
            tc.tile_pool(name="psl4", bufs=1, space="PSUM") as psl4,
            tc.tile_pool(name="pscb", bufs=2, space="PSUM") as pscb,
            tc.tile_pool(name="dram", bufs=1, space="DRAM") as dram,
        ):
            # dma_start ISSUE costs ~0.6us on a sequencer, serially.  Only
            # sync has the fast HWDGE path for bulk; the ACT sequencer is idle
            # for the first ~15us, so it issues the const/weight DMAs, letting
            # sync start on x immediately.
            early = [0]

            def dma_rr(out, in_):
                if early[0] > 0:
                    early[0] -= 1
                    nc.scalar.dma_start(out=out, in_=in_)
                else:
                    nc.sync.dma_start(out=out, in_=in_)

            # bias/constant tile first (a late bias DMA gates every GELU on
            # the in-order ACT queue); one packed DMA.
            early[0] = 12  # cst + w1f quarters + w4 + w2f halves + w3f + gw1/2/3
            cst = const.tile([128, 23], f32, tag="cst")
            dma_rr(cst[:], cst_d[:, :])
            b1 = cst[:, 0:8]
            b2 = cst[:, 8:12]
            b3 = cst[:, 12:14]
            gb1 = cst[:, 14:18]
            gb2 = cst[:, 18:20]
            b4h = cst[0:1, 20:21]   # b4 / 2 (host-packed)
            gb3h = cst[0:1, 21:22]  # gb3 / 2 (host-packed)
            ar = cst[0:1, 22:23]

            # coupling-net weights; split across DMAs for channel parallelism
            # (~85 GB/s per channel)
            w1f = const.tile([128, 8, 8, 128], f8, tag="w1f")  # [p, fo, s, j]
            for h in range(4):
                dma_rr(w1f[:, 2 * h:2 * h + 2, :, :],
                       w1_d[:, 2 * h:2 * h + 2, :, :])
            w4f = const.tile([128, 2, 1], f8, tag="w4f")
            dma_rr(w4f[:], w4_d[:, :, :])
            w2f = const.tile([128, 4, 8, 128], f8, tag="w2f")
            for h in range(2):
                dma_rr(w2f[:, 2 * h:2 * h + 2, :, :],
                       w2_d[:, 2 * h:2 * h + 2, :, :])
            w3f = const.tile([128, 2, 4, 128], f8, tag="w3f")
            dma_rr(w3f[:], w3_d[:, :, :, :])
            gw1 = const.tile([128, 8, 512], bf16, tag="gw1")  # [p, k, m]
            dma_rr(gw1[:], gw1_d[:, :, :])
            gw2 = const.tile([128, 4, 256], bf16, tag="gw2")
            dma_rr(gw2[:], gw2_d[:, :, :])
            gw3 = const.tile([128, 2, 1], bf16, tag="gw3")
            dma_rr(gw3[:], gw3_d[:, :, :])

            # x carrier (bf16, updated in place) + cycle-0 fp8 moving operand;
            # two DMAs per chunk each (slab halves -> separate HW channels),
            # interleaved so chunk 0 lands first.
            xc = []   # [128, slab, tok] bf16 — `out` lives here
            x8c = []  # [128, slab, tok] fp8
            for c in range(NCHUNK):
                x8t = const.tile([128, 8, CHUNK], f8, tag=f"x8_{c}")
                xt = const.tile([128, 8, CHUNK], bf16, tag=f"xc_{c}")
                nsplit = 4 if c == 0 else 2  # chunk 0 gates the first matmul
                w = 8 // nsplit
                for h in range(nsplit):
                    sl = slice(w * h, w * h + w)
                    nc.sync.dma_start(out=x8t[:, sl, :], in_=x8_d[:, c, sl, :])
                for h in range(2):
                    sl = slice(4 * h, 4 * h + 4)
                    nc.sync.dma_start(out=xt[:, sl, :], in_=x_d[:, c, sl, :])
                xc.append(xt)
                x8c.append(x8t)

            def out_bf(t, c):
                return xc[c][:, t, :]

            # per-chunk tanh tiles + coupling-broadcast operands.  The
            # coupling affine c0' + cmul'*th rides the PE broadcast as TWO
            # accumulating K=1 matmuls (engines cannot write partition 1, so
            # a single K=2 stationary cannot be built from runtime scalars):
            #   cb = (cmul'*ones) (x) th  +  (c0'*ones) (x) ones_row
            th_t = []
            for c in range(NCHUNK):
                th = const.tile([1, CHUNK], bf16, tag=f"th_{c}")
                th_t.append(th)
            cm_row = const.tile([1, 128], bf16, tag="cm_row")
            c0_row = const.tile([1, 128], bf16, tag="c0_row")
            ones = const.tile([1, 128], bf16, tag="ones")
            nc.vector.memset(ones[:], 1.0)
            ones_row = const.tile([1, CHUNK], bf16, tag="ones_row")
            nc.vector.memset(ones_row[:], 1.0)

            # ---------------- global-context partial sums + AllReduce -------
            # per-(tile, chunk) partial sums on DVE (idle during cycle 0);
            # emitted per-chunk so the in-order queue never blocks on later x.
            red = const.tile([128, 8, NCHUNK], f32, tag="gred")

            def reduce_chunk(c):
                for t in range(8):
                    nc.vector.tensor_reduce(
                        red[:, t, c:c + 1], out_bf(t, c), axis=AX.X, op=ALU.add
                    )

            gs = const.tile([128, 8], f32, tag="gs")

            def finish_gsum():
                for t in range(8):
                    nc.vector.tensor_reduce(
                        gs[:, t:t + 1], red[:, t, :], axis=AX.X, op=ALU.add
                    )

            cc_in = dram.tile([128, 8], f32, tag="cc_in")
            cc_out = dram.tile([128, 8], f32, tag="cc_out")
            gmean_f = const.tile([128, 8], f32, tag="gmean_f")
            gmean = const.tile([128, 8], bf16, tag="gmean")

            def do_collective():
                nc.sync.dma_start(out=cc_in[:], in_=gs[:])
                nc.gpsimd.collective_compute(
                    "AllReduce",
                    ALU.add,
                    ins=[cc_in.opt()],
                    outs=[cc_out.opt()],
                    replica_groups=[[0, 1], [2, 3], [4, 5], [6, 7]],
                )
                nc.sync.dma_start(out=gmean_f[:], in_=cc_out[:])
                nc.vector.tensor_copy(gmean[:], gmean_f[:])

            # ---------------- global net (emitted via hooks; see cycle 0) ---
            gc_tiles = {}

            def gc_stage1():
                # all 4 output-tile groups accumulate into one PSUM bank
                # (disjoint columns) -> a single GELU epilogue
                ps = psx.tile([128, 4], f32, tag="aux")
                for fo in range(4):
                    for k in range(8):
                        nc.tensor.matmul(
                            ps[:, fo:fo + 1],
                            gw1[:, k, fo * 128:(fo + 1) * 128],
                            gmean[:, k:k + 1], start=(k == 0), stop=(k == 7),
                        )
                # psum holds gc_w1.T @ sum(x); fold the 1/S mean + bias on DVE
                # (activation bias APs must be [P,1]; gb1 varies per column)
                z1 = work.tile([128, 4], f32, tag="z1")
                nc.vector.scalar_tensor_tensor(
                    z1[:], ps[:], 1.0 / S, gb1, ALU.mult, ALU.add
                )
                g1 = work.tile([128, 4], bf16, tag="g1")
                nc.scalar.activation(g1[:], z1[:], AF.Gelu)
                gc_tiles["g1"] = g1

            def gc_stage2():
                g1 = gc_tiles["g1"]
                ps = psx.tile([128, 2], f32, tag="aux")
                for fo in range(2):
                    for k in range(4):
                        nc.tensor.matmul(
                            ps[:, fo:fo + 1],
                            gw2[:, k, fo * 128:(fo + 1) * 128],
                            g1[:, k:k + 1], start=(k == 0), stop=(k == 3),
                        )
                z2 = work.tile([128, 2], f32, tag="z2")
                nc.vector.tensor_add(z2[:], ps[:], gb2)
                g2 = work.tile([128, 2], bf16, tag="g2")
                nc.scalar.activation(g2[:], z2[:], AF.Gelu)
                gc_tiles["g2"] = g2

            def gc_stage3():
                g2 = gc_tiles["g2"]
                ps = psx.tile([1, 1], f32, tag="aux")
                for k in range(2):
                    nc.tensor.matmul(
                        ps[:], gw3[:, k, :], g2[:, k:k + 1],
                        start=(k == 0), stop=(k == 1)
                    )
                # gf_t = tanh((gz + gb3)/2); sigmoid folded into the affine
                gft = const.tile([1, 1], f32, tag="gft")
                nc.scalar.activation(gft[:], ps[:], AF.Tanh, bias=gb3h, scale=0.5)

                # coupling = c0' + cmul' * tanh((z4+b4)/2)
                #   cmul' = 0.3*ar ;  c0' = 0.1 + 0.7*ar*gf_t
                cmul = const.tile([1, 1], f32, tag="cmul")
                nc.vector.tensor_scalar(cmul[:], ar, 0.3, None, ALU.mult)
                tmp0 = const.tile([1, 1], f32, tag="tmp0")
                nc.vector.tensor_scalar(tmp0[:], gft[:], 0.7, None, ALU.mult)
                c0 = const.tile([1, 1], f32, tag="c0")
                nc.vector.tensor_tensor(tmp0[:], ar, tmp0[:], ALU.mult)
                nc.vector.tensor_scalar(
                    c0[:], tmp0[:], BASE_COUPLING, None, ALU.add)
                # broadcast the two scalars across 128 cols (partition 0)
                nc.vector.tensor_scalar(
                    cm_row[:], ones[:], cmul[:], None, ALU.mult)
                nc.vector.tensor_scalar(
                    c0_row[:], ones[:], c0[:], None, ALU.mult)

            # ---------------- per-chunk building blocks ----------------
            pending_xb = [x8c[c] for c in range(NCHUNK)]
            hstate = {}

            def emit_L1(c):
                xb = pending_xb[c]
                pending_xb[c] = None
                h1 = work.tile([128, 8, CHUNK], f8, tag="h1")
                for fo in range(8):
                    ps1 = psm.tile([128, CHUNK], f32, tag="mm")
                    for s in range(4):
                        nc.tensor.matmul(
                            ps1[:], w1f[:, fo, 2 * s:2 * s + 2, :],
                            xb[:, 2 * s:2 * s + 2, :],
                            start=(s == 0), stop=(s == 3), perf_mode=DR,
                        )
                    nc.scalar.activation(
                        h1[:, fo, :], ps1[:], AF.Gelu, bias=b1[:, fo:fo + 1])
                hstate[c] = h1

            def emit_L2(c):
                h1 = hstate[c]
                h2 = work.tile([128, 4, CHUNK], f8, tag="h2")
                for fo in range(4):
                    ps2 = psm.tile([128, CHUNK], f32, tag="mm")
                    for s in range(4):
                        nc.tensor.matmul(
                            ps2[:], w2f[:, fo, 2 * s:2 * s + 2, :],
                            h1[:, 2 * s:2 * s + 2, :],
                            start=(s == 0), stop=(s == 3), perf_mode=DR,
                        )
                    nc.scalar.activation(
                        h2[:, fo, :], ps2[:], AF.Gelu, bias=b2[:, fo:fo + 1])
                hstate[c] = h2

            def emit_L34(c):
                h2 = hstate.pop(c)
                h3 = work.tile([128, 2, CHUNK], f8, tag="h3")
                for fo in range(2):
                    ps3 = psm.tile([128, CHUNK], f32, tag="mm")
                    for s in range(2):
                        nc.tensor.matmul(
                            ps3[:], w3f[:, fo, 2 * s:2 * s + 2, :],
                            h2[:, 2 * s:2 * s + 2, :],
                            start=(s == 0), stop=(s == 1), perf_mode=DR,
                        )
                    nc.scalar.activation(
                        h3[:, fo, :], ps3[:], AF.Gelu, bias=b3[:, fo:fo + 1])
                # L4: M=1 forbids the DoubleRow ldweights layout -> 2 plain
                # fp8 matmuls (ISA check s3_lw_dual_fp8_restrictions)
                ps4 = psl4.tile([1, CHUNK], f32, tag="l4")
                for s in range(2):
                    nc.tensor.matmul(
                        ps4[:], w4f[:, s, :], h3[:, s, :],
                        start=(s == 0), stop=(s == 1),
                    )
                # th = tanh((z4 + b4)/2); sigmoid folded into the coupling
                nc.scalar.activation(
                    th_t[c][:], ps4[:], AF.Tanh, bias=b4h, scale=0.5)

            def mlp_chunk(c, hooks=()):
                """coupling-net MLP on chunk c of `out`.

                hooks: up to 3 closures emitted after L1/L2/L3+L4 — used to
                slot the previous chunks' coupling-broadcast + twist updates
                (and the tiny serial gc-net chain at cycle 0) into the queues
                at points where their ACT/DVE dependencies have had time to
                finish.
                """
                hooks = list(hooks) + [None] * 3
                emit_L1(c)
                if hooks[0]:
                    hooks[0]()
                emit_L2(c)
                if hooks[1]:
                    hooks[1]()
                emit_L34(c)
                if hooks[2]:
                    hooks[2]()

            def update_chunk(c, last, next_conv=False):
                """coupling broadcast + twist update (in place) on chunk c;
                one consolidated DMA out if last."""
                # cb[p, j] = cmul'*th[j] + c0'  via two accumulating K=1
                # matmuls (all operands on partition 0)
                cb = pscb.tile([128, CHUNK], f32, tag="cb")
                nc.tensor.matmul(
                    cb[:], cm_row[:], th_t[c][:, :], start=True, stop=False)
                nc.tensor.matmul(
                    cb[:], c0_row[:], ones_row[:], start=False, stop=True)
                # one bf16 SBUF copy so the twist tensor_tensor ops all run
                # in the DVE 2x_1p fast mode (PSUM/f32 operands disable it)
                cbb = work.tile([128, CHUNK], bf16, tag="cbb")
                nc.vector.tensor_copy(cbb[:], cb[:])
                xb_next = None
                for p in range(4):
                    t, u = p, p + 4
                    tmpa = work.tile([128, CHUNK], bf16, tag="twa")
                    tmpb = work.tile([128, CHUNK], bf16, tag="twb")
                    nc.vector.tensor_mul(tmpa[:], out_bf(u, c), cbb[:])
                    nc.vector.tensor_mul(tmpb[:], out_bf(t, c), cbb[:])
                    if TWIST_SIGN[t] > 0:
                        nc.vector.tensor_add(out_bf(t, c), out_bf(t, c), tmpa[:])
                    else:
                        nc.vector.tensor_sub(out_bf(t, c), out_bf(t, c), tmpa[:])
                    if TWIST_SIGN[u] > 0:
                        nc.vector.tensor_add(out_bf(u, c), out_bf(u, c), tmpb[:])
                    else:
                        nc.vector.tensor_sub(out_bf(u, c), out_bf(u, c), tmpb[:])
                    if next_conv:
                        # next cycle's fp8 conversion for this pair, emitted
                        # here so the in-order DVE queue releases the next
                        # chunk's matmul inputs as early as possible
                        if xb_next is None:
                            xb_next = xbp.tile([128, 8, CHUNK], f8, tag="xb")
                        nc.vector.tensor_copy(xb_next[:, t, :], out_bf(t, c))
                        nc.vector.tensor_copy(xb_next[:, u, :], out_bf(u, c))
                    if last and p % 2 == 1:
                        # DMA out slabs {p-1, p, p+3, p+4} as soon as their
                        # twist adds retire (shrinks the end-of-kernel tail)
                        for tt in (p - 1, p):
                            nc.sync.dma_start(
                                out=out_d[:, c, tt:tt + 4 + 1:4, :],
                                in_=xc[c][:, tt:tt + 4 + 1:4, :],
                            )
                if next_conv and not last:
                    pending_xb[c] = xb_next

            # ---------------- main cycles ----------------
            # Cycle 0, chunks 0+1 layer-paired: the in-order PE queue would
            # otherwise stall L1(1) behind L2(0)'s wait for chunk-0's trailing
            # GELU at the very start (ACT lags the first chunk's matmuls).
            # Reductions are emitted eagerly; the collective is issued as soon
            # as chunk 3's x lands; the serial gc-net stages ride mlp(2)'s
            # hooks (gmean arrives ~30us, well before); cycle-0 updates then
            # ride mlp(3)'s hooks so cycle-1 L1s start with zero boundary
            # stall.  From there every mlp hook emits the oldest pending
            # update (lag-2 at the boundary, catching back to lag-1 in cycle
            # 1 so the tail stays one update deep).
            emit_L1(0)
            reduce_chunk(0)
            emit_L1(1)
            reduce_chunk(1)
            reduce_chunk(2)
            reduce_chunk(3)
            finish_gsum()
            do_collective()
            emit_L2(0)
            emit_L2(1)
            emit_L34(0)
            emit_L34(1)
            mlp_chunk(2, hooks=(gc_stage1, gc_stage2, gc_stage3))

            def upd(c, last=False):
                return lambda: update_chunk(c, last, next_conv=not last)

            mlp_chunk(3, hooks=(upd(0), None, upd(1)))
            # cycle 1: catch from lag-2 back to lag-1
            mlp_chunk(0, hooks=(upd(2),))
            mlp_chunk(1, hooks=(upd(3), None, upd(0)))
            mlp_chunk(2, hooks=(upd(1),))
            mlp_chunk(3, hooks=(upd(2),))
            # cycle 2 (last): updates write the final output + DMA out
            mlp_chunk(0, hooks=(upd(3),))
            mlp_chunk(1, hooks=(upd(0, last=True),))
            mlp_chunk(2, hooks=(upd(1, last=True),))
            mlp_chunk(3, hooks=(upd(2, last=True),))
            update_chunk(3, last=True, next_conv=False)

    nc.compile()
    return nc


def _get_graph():
    if "nc" not in _CACHE:
        _CACHE["nc"] = _build_graph()
    return _CACHE["nc"]


def _pack_consts(inputs):
    cst = np.zeros((128, 23), np.float32)
    cst[:, 0:8] = np.asarray(inputs["cn_b1"], np.float32).reshape(8, 128).T
    cst[:, 8:12] = np.asarray(inputs["cn_b2"], np.float32).reshape(4, 128).T
    cst[:, 12:14] = np.asarray(inputs["cn_b3"], np.float32).reshape(2, 128).T
    cst[:, 14:18] = np.asarray(inputs["gc_b1"], np.float32).reshape(4, 128).T
    cst[:, 18:20] = np.asarray(inputs["gc_b2"], np.float32).reshape(2, 128).T
    cst[0, 20] = 0.5 * np.asarray(inputs["cn_b4"], np.float32).reshape(())
    cst[0, 21] = 0.5 * np.asarray(inputs["gc_b3"], np.float32).reshape(())
    cst[0, 22] = np.asarray(inputs["adaptive_range"], np.float32).reshape(())
    return cst


def _make_in_maps(inputs):
    import ml_dtypes

    bf = ml_dtypes.bfloat16
    f8 = ml_dtypes.float8_e4m3
    x = np.ascontiguousarray(inputs["x"], dtype=np.float32)
    # [core, p, chunk, slab, tok] — exact SBUF tile layout, so every DMA is
    # partition-contiguous: x[b, s, d] with token j = chunk*512+tok,
    # feature f = slab*128+p
    xs = (x.reshape(NCORES, NCHUNK, CHUNK, 8, 128)
          .transpose(0, 4, 1, 3, 2))  # [8, 128, 4, 8, 512]
    xs = np.ascontiguousarray(xs)

    def pack_w(w, nfo):
        # [s*128+p, fo*128+j] -> [p, fo, s, j]
        kin = w.shape[0]
        return np.ascontiguousarray(
            np.asarray(w).reshape(kin // 128, 128, nfo, 128)
            .transpose(1, 2, 0, 3), dtype=f8)

    def pack_gw(w):
        # [k*128+p, m] -> [p, k, m]
        kin, m = w.shape
        return np.ascontiguousarray(
            np.asarray(w).reshape(kin // 128, 128, m).transpose(1, 0, 2),
            dtype=bf)

    shared = {
        "cn_w1": pack_w(inputs["cn_w1"], 8),
        "cn_w2": pack_w(inputs["cn_w2"], 4),
        "cn_w3": pack_w(inputs["cn_w3"], 2),
        "cn_w4": np.ascontiguousarray(
            np.asarray(inputs["cn_w4"]).reshape(2, 128).T.reshape(128, 2, 1),
            dtype=f8),
        "gc_w1": pack_gw(np.asarray(inputs["gc_w1"])),
        "gc_w2": pack_gw(np.asarray(inputs["gc_w2"])),
        "gc_w3": pack_gw(np.asarray(inputs["gc_w3"]).reshape(256, 1)),
        "consts": _pack_consts(inputs),
    }
    in_maps = []
    for c in range(NCORES):
        m = dict(shared)
        m["x"] = xs[c].astype(bf)
        m["x8"] = xs[c].astype(f8)
        in_maps.append(m)
    return in_maps


def _run(inputs, trace=False):
    from concourse.bass_utils import run_bass_kernel_spmd

    nc = _get_graph()
    in_maps = _make_in_maps(inputs)
    res = run_bass_kernel_spmd(
        nc, in_maps, core_ids=list(range(NCORES)), trace=trace
    )
    # out[p, chunk, slab, tok] -> [token = chunk*512+tok, feat = slab*128+p]
    outs = np.stack(
        [np.asarray(res.results[c]["out"]).astype(np.float32)
         .transpose(1, 3, 2, 0).reshape(TOK, DIM)
         for c in range(NCORES)], axis=0
    )  # [8, 2048, 1024]
    full = outs.reshape(B, S, DIM).astype(np.float32)
    return full, res


def kernel(**inputs) -> np.ndarray:
    out, _ = _run(inputs, trace=False)
    return out
